# revision 4
# baseline (speedup 1.0000x reference)
"""GATv2 (nn_GATv2_49108656062978) Trainium2 Bass kernel, 8 NeuronCores SPMD.

v2 design (DMA-subsystem-aware rewrite of the dst-partitioned kernel):
  - Nodes partitioned by dst across 8 cores; each core builds the full
    [xl_eff | xs] fp16 table (25.7 MB) in its HBM from bf16 inputs, then
    gathers per-edge rows and runs the edge pipeline per 128-node bucket.
  - Self-loops never enter the gather: xl_own/xr/xs_own come from a per-
    bucket matmul (phase X), contributing score_self / xs_own directly.
  - Flexible-band split: gather-low reaches table rows [0, 32768), gather-
    high reaches [TR-32768, TR). Rows in the overlap band are assigned to
    either gather per (node) to balance jl ~ jh, so sum_b(JL+JH) is near
    the unified lower bound (~1.03x) instead of the fixed-half ~1.2x.
  - Gathers ride SWDGE queues 1-3 (parallel descriptor-gen Q7 cores;
    queue 0 generates synchronously on the Pool engine - avoided), issued
    PRE buckets ahead so DGE overlaps the bucket compute.
  - Phase T streams x in ~0.9 MB loads on the sync ring and stores table
    chunks in 1 MB groups on the scalar ring (two parallel HWDGE rings).
Host does graph partitioning / index prep / weight folding, and the final
unpermute. All FLOPs of the module run on device.
"""
import sys

sys.path.insert(0, "/opt/trn_rl_repo")

import numpy as np

import concourse.bass as bass
import concourse.bacc as bacc
import concourse.tile as tile
from concourse import mybir
from concourse.bass_utils import run_bass_kernel_spmd

N = 50000
F = 128
H = 4
C = 32
HC = H * C
NEG = 0.2
NCORES = 8
NPC = N // NCORES          # 6250 nodes per core
NB = (NPC + 127) // 128    # 49 buckets
NPAD = NB * 128            # 6272
TR = NCORES * NPAD         # 50176 table rows
NCH = TR // 128            # 392 chunks
REACH = 32768              # int16 gather reach in rows
HIBASE = TR - REACH        # 17408: high gather's base row
G = 4                      # table chunks per PSUM group
NLOAD = 7                  # xtab loads (56 chunks each; must divide by 2*G)
PRE = 3                    # gather dispatch look-ahead (buckets)
GBUFS = PRE + 1            # g-tile ring depth

f32 = mybir.dt.float32
f16 = mybir.dt.float16
bf16 = mybir.dt.bfloat16
i16 = mybir.dt.int16

LAST_RESULT = None
RUN_KWARGS = {}

GQ = [1, 2, 3]             # SWDGE queues for gathers


def _pack16(v: np.ndarray) -> np.ndarray:
    """int index stream -> dma_gather int16 layout [128, n/16]."""
    assert len(v) % 16 == 0
    t = v.reshape(-1, 16).T.astype(np.int16)
    return np.tile(t, (8, 1))


def _prep(x, edge_index, Wl, bl, Wr, br, Ws, bs, att, bias):
    src = edge_index[0].astype(np.int64)
    dst = edge_index[1].astype(np.int64)
    trow = (src // NPC) * NPAD + (src % NPC)
    owner = dst // NPC

    # ---- weights / att folding ----
    aflat = att.reshape(HC)
    colperm = []
    Ph = []
    for h in range(H):
        a_h = aflat[h * C:(h + 1) * C]
        pos = np.where(a_h > 0)[0]
        neg = np.where(a_h <= 0)[0]
        colperm += list(h * C + pos) + list(h * C + neg)
        Ph.append(int(len(pos)))
    colperm = np.array(colperm)
    aab = np.abs(aflat)[colperm].astype(np.float32)
    Wl_eff = aab[:, None] * Wl[colperm]
    bl_eff = aab * bl[colperm]
    Wr_eff = aab[:, None] * Wr[colperm]
    br_eff = aab * br[colperm]

    # xs stored c-major so the alpha-weighting multiply is innermost-
    # contiguous on both operands (2x DVE mode).
    cmaj = np.array([(k % H) * C + k // H for k in range(HC)])
    Ws_cm = Ws[cmaj]
    # biases: bl_eff+br_eff ride on the self/edge E-sum; bs rides on the
    # output bias (softmax weights sum to 1).
    w_all = np.ascontiguousarray(
        np.concatenate([Wr_eff.T, Wl_eff.T, Ws_cm.T], axis=1))       # [F,384]
    brl_rep = np.tile((br_eff + bl_eff)[None, :], (128, 1)).astype(np.float32)
    bout_rep = np.tile((bias + bs)[cmaj][None, :], (128, 1)).astype(np.float32)

    # ---- xtab (same for all cores): x rows in table order, transposed
    xtab = np.zeros((TR, F), np.float32)
    for r in range(NCORES):
        xtab[r * NPAD:r * NPAD + NPC] = x[r * NPC:(r + 1) * NPC]
    xtab_t = np.ascontiguousarray(xtab.T, dtype=np.float32)          # [F, TR]

    # ---- per-core partitioning with flexible-band low/high balance ----
    percore = []
    JLs = np.zeros((NCORES, NB), np.int64)
    JHs = np.zeros((NCORES, NB), np.int64)
    for r in range(NCORES):
        sel = owner == r
        s_r = trow[sel]
        d_r = dst[sel] - r * NPC
        # band classes: 0 must-low, 1 flexible, 2 must-high
        cls = np.where(s_r < HIBASE, 0, np.where(s_r >= REACH, 2, 1))
        T = np.bincount(d_r, minlength=NPAD)
        A = np.bincount(d_r[cls == 0], minlength=NPAD)
        B = np.bincount(d_r[cls == 1], minlength=NPAD)
        # per node: x of B goes low so L = A + x ~ ceil(T/2)
        xq = np.clip((T + 1) // 2 - A, 0, B)
        Lc = A + xq
        Hcnt = T - Lc
        # low-flag per edge: order flexible edges per node, first xq low
        ordf = np.lexsort((cls, d_r))   # by node, must-low < flex < must-high
        s_o = s_r[ordf]
        d_o = d_r[ordf]
        cls_o = cls[ordf]
        start = np.zeros(NPAD + 1, np.int64)
        start[1:] = np.cumsum(T)
        posn = np.arange(len(d_o)) - start[d_o]
        lowf = posn < Lc[d_o]            # first Lc edges of node are low
        assert np.all(lowf[cls_o == 0]) and not np.any(lowf[cls_o == 2])

        order = np.lexsort((-(Lc - Hcnt), -np.maximum(Lc, Hcnt)))
        JLs[r] = Lc[order].reshape(NB, 128).max(1)
        JHs[r] = Hcnt[order].reshape(NB, 128).max(1)
        percore.append((order, Lc, Hcnt, s_o, d_o, lowf))
    JL = JLs.max(0)
    JH = JHs.max(0)

    # ---- per-core slot buffers ----
    in_maps = []
    orders = []
    JLmax = max(int(JL.max()), 1)
    JHmax = max(int(JH.max()), 1)
    for r in range(NCORES):
        order, Lc, Hcnt, s_o, d_o, lowf = percore[r]
        orders.append(order)
        bp = np.empty(NPAD, np.int64)          # node -> bucket position
        bp[order] = np.arange(NPAD)

        AL = np.zeros((NPAD, JLmax), np.int64)
        AH = np.zeros((NPAD, JHmax), np.int64)
        ML = np.zeros((NPAD, JLmax), np.float16)
        MH = np.zeros((NPAD, JHmax), np.float16)
        # per-node running position among low / high edges
        startL = np.zeros(NPAD + 1, np.int64)
        startL[1:] = np.cumsum(Lc)
        startH = np.zeros(NPAD + 1, np.int64)
        startH[1:] = np.cumsum(Hcnt)
        sl = s_o[lowf]
        dl = d_o[lowf]
        posl = np.arange(len(dl)) - startL[dl]
        AL[bp[dl], posl] = sl
        ML[bp[dl], posl] = 1.0
        sh = s_o[~lowf] - HIBASE
        dh = d_o[~lowf]
        posh = np.arange(len(dh)) - startH[dh]
        AH[bp[dh], posh] = sh
        MH[bp[dh], posh] = 1.0

        lowvals, highvals, masks = [], [], []
        for b in range(NB):
            jl, jh = int(JL[b]), int(JH[b])
            rs = slice(b * 128, (b + 1) * 128)
            lowvals.append(AL[rs, :jl].T.reshape(-1))     # j-major positions
            highvals.append(AH[rs, :jh].T.reshape(-1))
            masks.append(np.concatenate([ML[rs, :jl], MH[rs, :jh]], axis=1))
        lv = np.concatenate(lowvals) if lowvals else np.zeros(0, np.int64)
        hv = np.concatenate(highvals) if highvals else np.zeros(0, np.int64)
        maskall = np.ascontiguousarray(np.concatenate(masks, axis=1))

        xperm = np.zeros((NPAD, F), np.float32)
        xperm[order < NPC] = x[r * NPC:(r + 1) * NPC][order[order < NPC]]
        xperm_t = np.ascontiguousarray(xperm.T)              # [F, NPAD]

        in_maps.append({
            "xtab_t": xtab_t,
            "xperm_t": xperm_t,
            "idxlo": _pack16(lv), "idxhi": _pack16(hv),
            "maskall": maskall,
            "w_all": w_all,
            "brl_rep": brl_rep, "bout_rep": bout_rep,
        })
    return in_maps, orders, JL, JH, Ph


def _build(JL, JH, Ph, ncols_lo, ncols_hi, ncols_mask):
    nc = bacc.Bacc("TRN2", target_bir_lowering=False, debug=False,
                   num_devices=NCORES, num_swdge_queues=4,
                   dynamic_dma_scratch_size=32768)
    add = mybir.AluOpType.add
    sub = mybir.AluOpType.subtract
    mult = mybir.AluOpType.mult
    amax = mybir.AluOpType.max

    xtab_d = nc.dram_tensor("xtab_t", [F, TR], bf16, kind="ExternalInput")
    xperm_d = nc.dram_tensor("xperm_t", [F, NPAD], bf16, kind="ExternalInput")
    idxlo_d = nc.dram_tensor("idxlo", [128, ncols_lo], i16, kind="ExternalInput")
    idxhi_d = nc.dram_tensor("idxhi", [128, ncols_hi], i16, kind="ExternalInput")
    mask_d = nc.dram_tensor("maskall", [128, ncols_mask], f16, kind="ExternalInput")
    w_all_d = nc.dram_tensor("w_all", [F, 384], bf16, kind="ExternalInput")
    brl_d = nc.dram_tensor("brl_rep", [128, HC], f32, kind="ExternalInput")
    bout_d = nc.dram_tensor("bout_rep", [128, HC], f32, kind="ExternalInput")

    table_d = nc.dram_tensor("table2", [TR, 256], f16)         # internal
    out_d = nc.dram_tensor("outp", [NPAD, HC], f32, kind="ExternalOutput")

    with nc.allow_low_precision(reason="fp16 edge pipeline; fp32 where it matters"), \
         tile.TileContext(nc) as tc:
        with (
            tc.tile_pool(name="const", bufs=1) as cpool,
            tc.tile_pool(name="tpool", bufs=2) as tpool,
            tc.tile_pool(name="gpool", bufs=GBUFS) as gpool,
            tc.tile_pool(name="spool", bufs=3) as spool,
            tc.tile_pool(name="psx", bufs=2, space="PSUM") as psxp,
            tc.tile_pool(name="pst", bufs=2, space="PSUM") as pstp,
        ):
            # ---- constants ----
            w_all_sb = cpool.tile([F, 384], bf16)
            nc.sync.dma_start(w_all_sb[:], w_all_d[:])
            brl_sb = cpool.tile([128, HC], f32)
            nc.sync.dma_start(brl_sb[:], brl_d[:])
            bout_sb = cpool.tile([128, HC], f32)
            nc.sync.dma_start(bout_sb[:], bout_d[:])
            idxlo_sb = cpool.tile([128, ncols_lo], i16)
            nc.sync.dma_start(idxlo_sb[:], idxlo_d[:])
            idxhi_sb = cpool.tile([128, ncols_hi], i16)
            nc.sync.dma_start(idxhi_sb[:], idxhi_d[:])
            mask_sb = cpool.tile([128, ncols_mask], f16)
            nc.sync.dma_start(mask_sb[:], mask_d[:])
            xperm_sb = cpool.tile([F, NPAD], bf16)
            nc.sync.dma_start(xperm_sb[:], xperm_d[:])
            xr_sb = cpool.tile([128, NB * 128], f16)
            xs_own_sb = cpool.tile([128, NB * 128], f16)
            scr_self_sb = cpool.tile([128, NB * H], f16)
            zero_sb = cpool.tile([128, HC], f32)
            nc.vector.memset(zero_sb[:], 0.0)

            need_memset_P = any(p == 0 for p in Ph)
            need_memset_N = any(p == C for p in Ph)

            # ---- phase X: per-bucket own-node transforms + self scores ----
            for b in range(NB):
                px = psxp.tile([128, 384], f32)
                nc.tensor.matmul(px[:], lhsT=xperm_sb[:, b * 128:(b + 1) * 128],
                                 rhs=w_all_sb[:], start=True, stop=True)
                xr_b = xr_sb[:, b * 128:(b + 1) * 128]
                nc.vector.tensor_tensor(out=xr_b, in0=px[:, 0:HC],
                                        in1=brl_sb[:], op=add)
                # E_self = xl_own + (xr + brl)  [reuse xr_b which has brl]
                es = spool.tile([128, HC], f16, tag="es")
                nc.vector.tensor_tensor(out=es[:], in0=px[:, HC:2 * HC],
                                        in1=xr_b, op=add)
                nc.scalar.activation(es[:], es[:],
                                     mybir.ActivationFunctionType.Prelu,
                                     alpha=NEG)
                sp = spool.tile([128, H], f16, tag="sp")
                sn = spool.tile([128, H], f16, tag="sn")
                if need_memset_P:
                    nc.vector.memset(sp[:], 0.0)
                if need_memset_N:
                    nc.vector.memset(sn[:], 0.0)
                for h in range(H):
                    ph = Ph[h]
                    if ph > 0:
                        nc.vector.tensor_reduce(
                            out=sp[:, h:h + 1], in_=es[:, h * C:h * C + ph],
                            axis=mybir.AxisListType.X, op=add)
                    if ph < C:
                        nc.vector.tensor_reduce(
                            out=sn[:, h:h + 1],
                            in_=es[:, h * C + ph:(h + 1) * C],
                            axis=mybir.AxisListType.X, op=add)
                nc.vector.tensor_tensor(
                    out=scr_self_sb[:, b * H:(b + 1) * H],
                    in0=sp[:], in1=sn[:], op=sub)
                # xs_own (c-major already via Ws_cm in w_all); DVE TT —
                # PSUM-in + big-cpool-slice-out crashes other engines.
                nc.vector.tensor_tensor(
                    out=xs_own_sb[:, b * 128:(b + 1) * 128],
                    in0=px[:, 2 * HC:3 * HC], in1=zero_sb[:], op=add)
                del px

            # ---- phase T: full [xl_eff | xs] table ----
            # G=4-chunk PSUM groups (2 banks), 2 groups staged per 1 MB store
            table_v = table_d[:].rearrange("(a p) d -> p a d", p=128)
            CPL = NCH // NLOAD            # 28 chunks per load
            SG = 2 * G                    # chunks per store
            for i in range(NLOAD):
                xg = tpool.tile([128, CPL * 128], bf16, tag="xg")
                nc.sync.dma_start(
                    xg[:], xtab_d[:, i * CPL * 128:(i + 1) * CPL * 128])
                for si in range(CPL // SG):
                    tch = tpool.tile([128, SG, 256], f16, tag="tch")
                    for gi in range(2):
                        pt = pstp.tile([128, G * 256], f32)
                        for k in range(G):
                            kk = si * SG + gi * G + k
                            nc.tensor.matmul(
                                pt[:, k * 256:(k + 1) * 256],
                                lhsT=xg[:, kk * 128:(kk + 1) * 128],
                                rhs=w_all_sb[:, HC:384], start=True, stop=True)
                        nc.scalar.copy(
                            tch[:, gi * G:(gi + 1) * G, :].rearrange(
                                "p a d -> p (a d)"), pt[:])
                        del pt
                    g0 = i * CPL + si * SG
                    # alternate the two HWDGE rings so stores run ring-parallel
                    eng = nc.scalar if (i * (CPL // SG) + si) % 2 == 0 else nc.sync
                    eng.dma_start(table_v[:, g0:g0 + SG, :], tch[:])

            # ---- phase M: plain gathers, dispatched PRE buckets ahead so
            # the per-queue desc-gen overlaps bucket compute ----
            offs_lo = np.concatenate([[0], np.cumsum(JL * 128)]).astype(int)
            offs_hi = np.concatenate([[0], np.cumsum(JH * 128)]).astype(int)
            offs_m = np.concatenate([[0], np.cumsum(JL + JH)]).astype(int)
            gtiles = {}

            def issue_gather(b):
                jl, jh = int(JL[b]), int(JH[b])
                J = jl + jh
                if J == 0:
                    return
                g = gpool.tile([128, J, 256], f16, tag="g")
                gtiles[b] = g
                if jl:
                    nc.gpsimd.dma_gather(
                        out_ap=g[:, 0:jl, :], in_ap=table_d[0:REACH, :],
                        idxs_ap=idxlo_sb[:, offs_lo[b] // 16:
                                         (offs_lo[b] + jl * 128) // 16],
                        num_idxs=jl * 128, num_idxs_reg=jl * 128,
                        elem_size=256, queue_num=GQ[(2 * b) % 3],
                        single_packet=False)
                if jh:
                    # 2 of 3 high halves run Pool-synchronous desc-gen on
                    # queue 0 (8.2ns/idx) in parallel with queues 1-3's
                    # async processors (~18ns/idx each)
                    qh = 0 if b % 3 != 2 else GQ[(2 * b + 1) % 3]
                    nc.gpsimd.dma_gather(
                        out_ap=g[:, jl:J, :], in_ap=table_d[HIBASE:TR, :],
                        idxs_ap=idxhi_sb[:, offs_hi[b] // 16:
                                         (offs_hi[b] + jh * 128) // 16],
                        num_idxs=jh * 128, num_idxs_reg=jh * 128,
                        elem_size=256, queue_num=qh,
                        single_packet=False)

            for b in range(min(PRE, NB)):
                issue_gather(b)

            for b in range(NB):
                if b + PRE < NB:
                    issue_gather(b + PRE)
                jl, jh = int(JL[b]), int(JH[b])
                J = jl + jh
                xr_b = xr_sb[:, b * 128:(b + 1) * 128]
                xs_own_b = xs_own_sb[:, b * 128:(b + 1) * 128]
                scr_self_b = scr_self_sb[:, b * H:(b + 1) * H]

                if J == 0:
                    # self-loop only: alpha = 1 -> out = xs_own + bias
                    outb = spool.tile([128, HC], f32, tag="outb")
                    nc.vector.tensor_tensor(out=outb[:], in0=xs_own_b,
                                            in1=bout_sb[:], op=add)
                    nc.sync.dma_start(out_d[b * 128:(b + 1) * 128, :], outb[:])
                    continue

                g = gtiles.pop(b)
                # E = xl_g + xr (incl brl via xr_b? no: xr_b has brl folded)
                if jl:
                    nc.vector.tensor_tensor(
                        out=g[:, 0:jl, 0:HC], in0=g[:, 0:jl, 0:HC],
                        in1=xr_b.unsqueeze(1).broadcast_to([128, jl, HC]),
                        op=add)
                    nc.scalar.activation(g[:, 0:jl, 0:HC], g[:, 0:jl, 0:HC],
                                         mybir.ActivationFunctionType.Prelu,
                                         alpha=NEG)
                if jh:
                    nc.vector.tensor_tensor(
                        out=g[:, jl:J, 0:HC], in0=g[:, jl:J, 0:HC],
                        in1=xr_b.unsqueeze(1).broadcast_to([128, jh, HC]),
                        op=add)
                    nc.scalar.activation(g[:, jl:J, 0:HC], g[:, jl:J, 0:HC],
                                         mybir.ActivationFunctionType.Prelu,
                                         alpha=NEG)

                scrP = spool.tile([128, J, H], f16, tag="scrP")
                scrN = spool.tile([128, J, H], f16, tag="scrN")
                if need_memset_P:
                    nc.vector.memset(scrP[:], 0.0)
                if need_memset_N:
                    nc.vector.memset(scrN[:], 0.0)
                for h in range(H):
                    ph = Ph[h]
                    if ph > 0:
                        nc.vector.tensor_reduce(
                            out=scrP[:, :, h], in_=g[:, :, h * C:h * C + ph],
                            axis=mybir.AxisListType.X, op=add)
                    if ph < C:
                        nc.vector.tensor_reduce(
                            out=scrN[:, :, h],
                            in_=g[:, :, h * C + ph:(h + 1) * C],
                            axis=mybir.AxisListType.X, op=add)
                scr = spool.tile([128, J, H], f16, tag="scr")
                nc.vector.tensor_tensor(out=scr[:], in0=scrP[:], in1=scrN[:],
                                        op=sub)

                mx0 = spool.tile([128, H], f16, tag="mx0")
                nc.vector.tensor_reduce(
                    out=mx0[:], in_=scr[:].rearrange("p j h -> p h j"),
                    axis=mybir.AxisListType.X, op=amax)
                mx = spool.tile([128, H], f16, tag="mx")
                nc.vector.tensor_tensor(out=mx[:], in0=mx0[:], in1=scr_self_b,
                                        op=amax)
                msb = spool.tile([128, J, H], f16, tag="msb")
                nc.vector.tensor_tensor(
                    out=msb[:], in0=scr[:],
                    in1=mx[:].unsqueeze(1).broadcast_to([128, J, H]), op=sub)
                pex = spool.tile([128, J, H], f16, tag="pex")
                nc.scalar.activation(pex[:], msb[:],
                                     mybir.ActivationFunctionType.Exp)
                msf = spool.tile([128, H], f16, tag="msf")
                nc.vector.tensor_tensor(out=msf[:], in0=scr_self_b, in1=mx[:],
                                        op=sub)
                pxs = spool.tile([128, H], f16, tag="pxs")
                nc.scalar.activation(pxs[:], msf[:],
                                     mybir.ActivationFunctionType.Exp)
                pm = spool.tile([128, J, H], f16, tag="pm")
                nc.vector.tensor_tensor(
                    out=pm[:], in0=pex[:],
                    in1=mask_sb[:, offs_m[b]:offs_m[b] + J]
                        .unsqueeze(2).broadcast_to([128, J, H]),
                    op=mult)
                den0 = spool.tile([128, H], f16, tag="den0")
                nc.vector.tensor_reduce(
                    out=den0[:], in_=pm[:].rearrange("p j h -> p h j"),
                    axis=mybir.AxisListType.X, op=add)
                den = spool.tile([128, H], f16, tag="den")
                nc.vector.tensor_tensor(out=den[:], in0=den0[:], in1=pxs[:],
                                        op=add)

                # self-term accumulator: acc = pxs (*) xs_own  (c-major)
                acc = spool.tile([128, HC], f16, tag="acc")
                nc.vector.tensor_tensor(
                    out=acc[:].rearrange("p (c h) -> p c h", h=H),
                    in0=xs_own_b.rearrange("p (c h) -> p c h", h=H),
                    in1=pxs[:].unsqueeze(1).broadcast_to([128, C, H]),
                    op=mult)

                # weighted xs in place, then pairwise tree-sum over j
                def _wmul(j0, jn):
                    nc.vector.tensor_tensor(
                        out=g[:, j0:j0 + jn, HC:256].rearrange(
                            "p j (c h) -> p j c h", h=H),
                        in0=g[:, j0:j0 + jn, HC:256].rearrange(
                            "p j (c h) -> p j c h", h=H),
                        in1=pm[:, j0:j0 + jn, :].unsqueeze(2).broadcast_to(
                            [128, jn, C, H]),
                        op=mult)

                if jl:
                    _wmul(0, jl)
                if jh:
                    _wmul(jl, jh)
                n = J
                while n > 1:
                    k = n // 2
                    nc.vector.tensor_tensor(
                        out=g[:, 0:k, HC:256], in0=g[:, 0:k, HC:256],
                        in1=g[:, n - k:n, HC:256], op=add)
                    n = n - k
                agg = spool.tile([128, HC], f16, tag="agg")
                nc.vector.tensor_tensor(out=agg[:], in0=g[:, 0, HC:256],
                                        in1=acc[:], op=add)

                rd = spool.tile([128, H], f16, tag="rd")
                nc.vector.reciprocal(rd[:], den[:])
                outn = spool.tile([128, HC], f16, tag="outn")
                nc.vector.tensor_tensor(
                    out=outn[:].rearrange("p (c h) -> p c h", h=H),
                    in0=agg[:].rearrange("p (c h) -> p c h", h=H),
                    in1=rd[:].unsqueeze(1).broadcast_to([128, C, H]),
                    op=mult)
                outb = spool.tile([128, HC], f32, tag="outb")
                nc.vector.tensor_tensor(out=outb[:], in0=outn[:],
                                        in1=bout_sb[:], op=add)
                nc.sync.dma_start(out_d[b * 128:(b + 1) * 128, :], outb[:])

    import os
    if not os.environ.get("K2_BUILD_ONLY"):
        nc.compile()
    return nc


def kernel(**inputs) -> np.ndarray:
    global LAST_RESULT
    import ml_dtypes
    ins = {k: np.asarray(v) for k, v in inputs.items()}
    in_maps, orders, JL, JH, Ph = _prep(
        ins["x"].astype(np.float32), ins["edge_index"],
        ins["Wl"].astype(np.float32), ins["bl"].astype(np.float32),
        ins["Wr"].astype(np.float32), ins["br"].astype(np.float32),
        ins["Ws"].astype(np.float32), ins["bs"].astype(np.float32),
        ins["att"].astype(np.float32), ins["bias"].astype(np.float32))
    for m in in_maps:
        m["xtab_t"] = m["xtab_t"].astype(ml_dtypes.bfloat16)
        m["xperm_t"] = m["xperm_t"].astype(ml_dtypes.bfloat16)
        m["w_all"] = m["w_all"].astype(ml_dtypes.bfloat16)
    ncols_lo = in_maps[0]["idxlo"].shape[1]
    ncols_hi = in_maps[0]["idxhi"].shape[1]
    ncols_mask = in_maps[0]["maskall"].shape[1]
    nc = _build(JL, JH, Ph, ncols_lo, ncols_hi, ncols_mask)
    res = run_bass_kernel_spmd(nc, in_maps, core_ids=list(range(NCORES)),
                               **RUN_KWARGS)
    LAST_RESULT = res
    cmaj = np.array([(k % H) * C + k // H for k in range(HC)])
    inv = np.empty(HC, np.int64)
    inv[cmaj] = np.arange(HC)
    out = np.zeros((N, HC), np.float32)
    for r in range(NCORES):
        o = res.results[r]["outp"]
        valid = orders[r] < NPC
        out[r * NPC + orders[r][valid]] = o[np.where(valid)[0]][:, inv]
    return out
